# revision 3
# baseline (speedup 1.0000x reference)
"""DGCNN feature extractor on 8 Trainium2 NeuronCores (Bass/Tile) — v3.

Data-parallel over batch B=8 (one sample per core). Per edge-conv layer:
  - scores s[n,m] = <x_n,x_m> - |x_m|^2/2 via PE matmul with appended aug
    rows (ones row on the query side, bias row on the key side)
  - scalar engine copies score chunks PSUM->SBUF; vector topk (chunked
    max8 -> top-16 -> 2x max_index) reads SBUF
  - neighbor fetch BATCHED: one SWDGE dma_gather per 4 point-tiles
    (8192 rows; 2 tiles for layer 3) from a DRAM [v | v^2] table
    (f32 for layers 1/2, bf16 for layer 3), wrapped-index tile built
    on-device with 8 PE selection matmuls + scalar shuffles + one PE
    broadcast matmul; fold trees lag one batch so vector never waits
  - BN batch stats via AllReduce; x_next = relu(a*(u + max_k v) + b)
Final: channel max pool, concat, FC on device; host stacks per-core rows.
"""
import numpy as np

B, C0, N, KNB = 8, 3, 4096, 16
O1, O2, O3 = 32, 32, 64
NCORES = 8
EPS = 1e-5
NTOT = float(B * N * KNB)
NT = N // 128          # 32 point-tiles per layer
NCH = 16               # score chunks per row (4096/256)
CHK = N // NCH         # 256
NEG = -3.0e38

_cache: dict = {}


def _build(sim_single=False):
    import concourse.bacc as bacc
    import concourse.bass as bass
    import concourse.mybir as mybir
    import concourse.tile as tile
    from concourse.masks import make_identity

    f32 = mybir.dt.float32
    bf16 = mybir.dt.bfloat16
    u32 = mybir.dt.uint32
    i16 = mybir.dt.int16
    AO = mybir.AluOpType
    AF = mybir.ActivationFunctionType

    nc = bacc.Bacc("TRN2", target_bir_lowering=False, debug=False,
                   num_devices=1 if sim_single else NCORES)

    # ---- I/O ----
    x_in = nc.dram_tensor("x", [C0, N], f32, kind="ExternalInput")
    wuv_in = [None,
              nc.dram_tensor("wuv1", [C0, 2 * O1], f32, kind="ExternalInput"),
              nc.dram_tensor("wuv2", [O1, 2 * O2], f32, kind="ExternalInput"),
              nc.dram_tensor("wuv3", [O2, 2 * O3], f32, kind="ExternalInput")]
    gb_in = [None,
             nc.dram_tensor("gb1", [O1, 2], f32, kind="ExternalInput"),
             nc.dram_tensor("gb2", [O2, 2], f32, kind="ExternalInput"),
             nc.dram_tensor("gb3", [O3, 2], f32, kind="ExternalInput")]
    wfct_in = nc.dram_tensor("wfct", [128, 64], f32, kind="ExternalInput")
    bfc_in = nc.dram_tensor("bfc", [1, 64], f32, kind="ExternalInput")
    out_d = nc.dram_tensor("out", [1, 64], f32, kind="ExternalOutput")

    # ---- internal DRAM ----
    vtab = [None,
            nc.dram_tensor("vtab1", [N, 2 * O1], f32),
            nc.dram_tensor("vtab2", [N, 2 * O2], f32),
            nc.dram_tensor("vtab3", [N, 2 * O3], bf16)]
    cc_in = [None] + [nc.dram_tensor(f"ccin{l}", [o, 2], f32)
                      for l, o in ((1, O1), (2, O2), (3, O3))]
    cc_out = [None] + [nc.dram_tensor(f"ccout{l}", [o, 2], f32,
                                      addr_space="Shared")
                       for l, o in ((1, O1), (2, O2), (3, O3))]

    with tile.TileContext(nc) as tc:
        with (
            tc.tile_pool(name="big", bufs=1) as bigp,
            tc.tile_pool(name="lay", bufs=1) as layp,
            tc.tile_pool(name="work", bufs=3) as workp,
            tc.tile_pool(name="gpool", bufs=2) as gp,
            tc.tile_pool(name="wip", bufs=2) as wip,
            tc.tile_pool(name="scb", bufs=2) as scbp,
            tc.tile_pool(name="const", bufs=1) as constp,
        ):
            ident = constp.tile([128, 128], f32)
            make_identity(nc, ident[:])
            ones128 = constp.tile([128, 1], f32)
            nc.vector.memset(ones128[:], 1.0)
            zero128 = constp.tile([128, 1], f32)
            nc.vector.memset(zero128[:], 0.0)
            eps128 = constp.tile([128, 1], f32)
            nc.vector.memset(eps128[:], EPS)
            xg = constp.tile([128, 1], f32)
            # BRD[q, 16b+qq] = (q == qq): replicates a [16,*] block 8x
            brd = constp.tile([16, 128], f32)
            for b in range(8):
                nc.vector.tensor_copy(out=brd[:, 16 * b:16 * b + 16],
                                      in_=ident[0:16, 0:16])

            def layer(l, C, O, xq, xk, is_last):
                """xq/xk: [C+1, N] SBUF tiles, rows 0..C-1 = x, row C = aug."""
                BT = 4          # tiles per gather batch
                vdt = bf16 if l == 3 else f32
                wuv = constp.tile([C, 2 * O], f32, tag=f"wuv{l}")
                nc.sync.dma_start(out=wuv[:], in_=wuv_in[l].ap())
                gb = constp.tile([O, 2], f32, tag=f"gb{l}")
                nc.sync.dma_start(out=gb[:], in_=gb_in[l].ap())

                # --- phase A: aug rows, u/v matmuls, vtab ---
                augst = layp.tile([1, N], f32, tag="augst")
                nc.vector.memset(augst[:], 1.0)
                nc.sync.dma_start(out=xq[C:C + 1, :], in_=augst[:])
                sqst = augst
                xsq = layp.tile([C, N], f32, tag="xsq")
                nc.scalar.activation(out=xsq[:], in_=xq[0:C, :],
                                     func=AF.Square, bias=zero128[0:C, :])
                onesC = constp.tile([C, 1], f32, tag=f"onesC{l}")
                nc.vector.memset(onesC[:], 1.0)
                ubuf = layp.tile([128, NT, O], f32, tag="ubuf")
                with tc.tile_pool(name=f"psA{l}", bufs=2, space="PSUM") as psA:
                    for ch in range(8):
                        sl = slice(512 * ch, 512 * (ch + 1))
                        sq_ps = psA.tile([1, 512], f32, tag="sqps")
                        nc.tensor.matmul(out=sq_ps[:], lhsT=onesC[:],
                                         rhs=xsq[:, sl], start=True, stop=True)
                        nc.scalar.activation(out=sqst[:, sl], in_=sq_ps[:],
                                             func=AF.Copy, scale=-0.5)
                    nc.sync.dma_start(out=xk[C:C + 1, :], in_=sqst[:])
                    for t in range(NT):
                        tl = slice(128 * t, 128 * (t + 1))
                        uv_ps = psA.tile([128, 2 * O], f32, tag="uvps")
                        nc.tensor.matmul(out=uv_ps[:], lhsT=xq[0:C, tl],
                                         rhs=wuv[:], start=True, stop=True)
                        nc.scalar.activation(out=ubuf[:, t, :], in_=uv_ps[:, 0:O],
                                             func=AF.Copy)
                        vstage = workp.tile([128, 2 * O], vdt, tag="vstage")
                        nc.vector.tensor_copy(out=vstage[:, 0:O], in_=uv_ps[:, O:2 * O])
                        nc.scalar.activation(out=vstage[:, O:2 * O],
                                             in_=uv_ps[:, O:2 * O],
                                             func=AF.Square, bias=zero128[:, :])
                        nc.sync.dma_start(out=vtab[l].ap()[tl, :], in_=vstage[:])

                # --- phase B ---
                Dbuf = layp.tile([128, NT, O], f32, tag="Dbuf")
                GG = layp.tile([128, NT, 2 * O], f32, tag="GG")
                with (
                    tc.tile_pool(name=f"psB{l}", bufs=1, space="PSUM") as psB,
                    tc.tile_pool(name=f"psW{l}", bufs=2, space="PSUM") as psW,
                ):
                    def folds_for(bt, BTn, g4):
                        """reduce over k for a whole gather batch via strided APs.
                        g4: [128, BTn*16, 2*O]; writes Dbuf/GG tile rows."""
                        base = g4[:]
                        kv = bass.AP(base.tensor, base.offset,
                                     [base.ap[0], [16 * 2 * O, BTn], [1, O],
                                      [2 * O, 16]])
                        nc.vector.tensor_reduce(
                            out=Dbuf[:, bt * BTn:(bt + 1) * BTn, :], in_=kv,
                            axis=mybir.AxisListType.X, op=AO.max)
                        ks = bass.AP(base.tensor, base.offset,
                                     [base.ap[0], [16 * 2 * O, BTn], [1, 2 * O],
                                      [2 * O, 16]])
                        nc.vector.tensor_reduce(
                            out=GG[:, bt * BTn:(bt + 1) * BTn, :], in_=ks,
                            axis=mybir.AxisListType.X, op=AO.add)

                    pend = None     # (batch_start_tile, g4)
                    nbt = NT // BT
                    for bt in range(nbt):
                        idxs4 = wip.tile([128, BT * 16], u32, tag="idxs4")
                        for u in range(BT):
                            t = bt * BT + u
                            tl = slice(128 * t, 128 * (t + 1))
                            ssb = scbp.tile([128, N], f32, tag="ssb")
                            for h in range(2):
                                stile = psB.tile([128, 2048], f32, tag="stile")
                                for c4 in range(4):
                                    sl = slice(2048 * h + 512 * c4,
                                               2048 * h + 512 * (c4 + 1))
                                    nc.tensor.matmul(
                                        out=stile[:, 512 * c4:512 * (c4 + 1)],
                                        lhsT=xq[:, tl], rhs=xk[:, sl],
                                        start=True, stop=True)
                                nc.scalar.activation(
                                    out=ssb[:, 2048 * h:2048 * (h + 1)],
                                    in_=stile[:], func=AF.Copy)
                            cand = workp.tile([128, 8 * NCH], f32, tag="cand")
                            for ch in range(NCH):
                                nc.vector.max(out=cand[:, 8 * ch:8 * ch + 8],
                                              in_=ssb[:, CHK * ch:CHK * (ch + 1)])
                            t16 = workp.tile([128, 16], f32, tag="t16")
                            cand2 = workp.tile([128, 8 * NCH], f32, tag="cand2")
                            nc.vector.max(out=t16[:, 0:8], in_=cand[:])
                            nc.vector.match_replace(out=cand2[:],
                                                    in_to_replace=t16[:, 0:8],
                                                    in_values=cand[:], imm_value=NEG)
                            nc.vector.max(out=t16[:, 8:16], in_=cand2[:])
                            nc.vector.max_index(out=idxs4[:, 16 * u:16 * u + 8],
                                                in_max=t16[:, 0:8], in_values=ssb[:])
                            nc.vector.max_index(out=idxs4[:, 16 * u + 8:16 * u + 16],
                                                in_max=t16[:, 8:16], in_values=ssb[:])
                        # batched wrapped-index:
                        # wi[16b+q, 128u+8j+r] = idxs4[16r+q, 16u+j]
                        idxf4 = wip.tile([128, BT * 16], f32, tag="idxf4")
                        nc.vector.tensor_copy(out=idxf4[:], in_=idxs4[:])
                        mps4 = psW.tile([16, 8, BT * 16], f32, tag="mps4")
                        for r in range(8):
                            nc.tensor.matmul(out=mps4[:, r, :],
                                             lhsT=ident[:, 16 * r:16 * r + 16],
                                             rhs=idxf4[:], start=True, stop=True)
                        w16 = wip.tile([16, BT, 16, 8], f32, tag="w16")
                        for r in range(8):
                            nc.scalar.activation(out=w16[:, :, :, r],
                                                 in_=mps4[:, r, :], func=AF.Copy)
                        rep4 = psW.tile([128, BT * 128], f32, tag="rep4")
                        nc.tensor.matmul(out=rep4[:], lhsT=brd[:],
                                         rhs=w16[:, :, :, :], start=True, stop=True)
                        wi4 = wip.tile([128, BT * 128], i16, tag="wi4")
                        nc.vector.tensor_copy(out=wi4[:], in_=rep4[:])
                        g4 = gp.tile([128, BT * 16, 2 * O], vdt, tag="g4")
                        nc.gpsimd.dma_gather(g4[:], vtab[l].ap(), wi4[:],
                                             BT * 2048, BT * 2048, 2 * O,
                                             single_packet=False)
                        if pend is not None:
                            folds_for(pend[0], BT, pend[1])
                        pend = (bt, g4)
                    folds_for(pend[0], BT, pend[1])

                # --- phase C: stats, allreduce, x_next ---
                tmp2 = layp.tile([128, NT, O], f32, tag="tmp2")
                nc.vector.tensor_tensor(out=tmp2[:], in0=ubuf[:], in1=GG[:, :, 0:O],
                                        op=AO.mult)
                nc.vector.tensor_scalar(tmp2[:], tmp2[:], 2.0, None, op0=AO.mult)
                nc.vector.tensor_tensor(out=GG[:, :, O:2 * O], in0=GG[:, :, O:2 * O],
                                        in1=tmp2[:], op=AO.add)
                nc.vector.tensor_tensor(out=tmp2[:], in0=ubuf[:], in1=ubuf[:],
                                        op=AO.mult)
                nc.vector.tensor_scalar(tmp2[:], tmp2[:], 16.0, None, op0=AO.mult)
                nc.vector.tensor_tensor(out=GG[:, :, O:2 * O], in0=GG[:, :, O:2 * O],
                                        in1=tmp2[:], op=AO.add)
                nc.vector.tensor_scalar(tmp2[:], ubuf[:], 16.0, None, op0=AO.mult)
                nc.vector.tensor_tensor(out=GG[:, :, 0:O], in0=GG[:, :, 0:O],
                                        in1=tmp2[:], op=AO.add)
                for h in (16, 8, 4, 2, 1):
                    nc.vector.tensor_tensor(out=GG[:, 0:h, :], in0=GG[:, 0:h, :],
                                            in1=GG[:, h:2 * h, :], op=AO.add)
                with tc.tile_pool(name=f"psR{l}", bufs=1, space="PSUM") as psR:
                    s1_ps = psR.tile([O, 1], f32, tag="s1ps")
                    s2_ps = psR.tile([O, 1], f32, tag="s2ps")
                    nc.tensor.matmul(out=s1_ps[:], lhsT=GG[:, 0, 0:O], rhs=ones128[:],
                                     start=True, stop=True)
                    nc.tensor.matmul(out=s2_ps[:], lhsT=GG[:, 0, O:2 * O],
                                     rhs=ones128[:], start=True, stop=True)
                    stg = workp.tile([O, 2], f32, tag="stg")
                    nc.vector.tensor_copy(out=stg[:, 0:1], in_=s1_ps[:])
                    nc.vector.tensor_copy(out=stg[:, 1:2], in_=s2_ps[:])
                    nc.sync.dma_start(out=cc_in[l].ap(), in_=stg[:])
                if sim_single:
                    nc.sync.dma_start(out=cc_out[l].ap(), in_=cc_in[l].ap())
                else:
                    nc.gpsimd.collective_compute(
                        "AllReduce", AO.add, replica_groups=[list(range(NCORES))],
                        ins=[cc_in[l].ap()], outs=[cc_out[l].ap()])
                stats = workp.tile([O, 2], f32, tag="stats")
                nc.sync.dma_start(out=stats[:], in_=cc_out[l].ap())
                mean = workp.tile([O, 4], f32, tag="mean")
                nc.vector.tensor_scalar(mean[:, 0:1], stats[:, 0:1], 1.0 / NTOT,
                                        None, op0=AO.mult)
                nc.vector.tensor_scalar(mean[:, 1:2], stats[:, 1:2], 1.0 / NTOT,
                                        None, op0=AO.mult)
                nc.vector.tensor_tensor(out=mean[:, 2:3], in0=mean[:, 0:1],
                                        in1=mean[:, 0:1], op=AO.mult)
                nc.vector.tensor_sub(mean[:, 1:2], mean[:, 1:2], mean[:, 2:3])
                nc.scalar.activation(out=mean[:, 1:2], in_=mean[:, 1:2],
                                     func=AF.Sqrt, bias=eps128[0:O, :])
                ab = workp.tile([O, 2], f32, tag="ab")
                nc.vector.reciprocal(out=ab[:, 0:1], in_=mean[:, 1:2])
                nc.vector.tensor_tensor(out=ab[:, 0:1], in0=ab[:, 0:1],
                                        in1=gb[:, 0:1], op=AO.mult)
                nc.vector.tensor_tensor(out=mean[:, 3:4], in0=mean[:, 0:1],
                                        in1=ab[:, 0:1], op=AO.mult)
                nc.vector.tensor_sub(ab[:, 1:2], gb[:, 1:2], mean[:, 3:4])

                nc.vector.tensor_tensor(out=Dbuf[:], in0=Dbuf[:], in1=ubuf[:],
                                        op=AO.add)
                if is_last:
                    xnq = bigp.tile([O3 + 1, N], f32, tag="xq")
                    xnk = None
                else:
                    xnq = bigp.tile([O + 1, N], f32, tag="xq")
                    xnk = bigp.tile([O + 1, N], f32, tag="xk")
                with tc.tile_pool(name=f"psT{l}", bufs=4, space="PSUM") as psT:
                    for t in range(NT):
                        tl = slice(128 * t, 128 * (t + 1))
                        tp = psT.tile([O, 128], f32, tag="tpps")
                        nc.tensor.transpose(out=tp[:], in_=Dbuf[:, t, :],
                                            identity=ident[:])
                        nc.scalar.activation(out=xnq[0:O, tl], in_=tp[:],
                                             func=AF.Relu,
                                             bias=ab[:, 1:2], scale=ab[:, 0:1])
                if xnk is not None:
                    nc.scalar.activation(out=xnk[0:O, :], in_=xnq[0:O, :],
                                         func=AF.Copy)
                cm = workp.tile([O, 1], f32, tag="cm")
                nc.vector.tensor_reduce(out=cm[:], in_=xnq[0:O, :],
                                        axis=mybir.AxisListType.X, op=AO.max)
                off = {1: 0, 2: O1, 3: O1 + O2}[l]
                nc.sync.dma_start(out=xg[off:off + O, :], in_=cm[:])
                return xnq, xnk

            xq1 = bigp.tile([C0 + 1, N], f32, tag="xq")
            xk1 = bigp.tile([C0 + 1, N], f32, tag="xk")
            nc.sync.dma_start(out=xq1[0:C0, :], in_=x_in.ap())
            nc.vector.tensor_copy(out=xk1[0:C0, :], in_=xq1[0:C0, :])

            xq2, xk2 = layer(1, C0, O1, xq1, xk1, False)
            xq3, xk3 = layer(2, O1, O2, xq2, xk2, False)
            layer(3, O2, O3, xq3, xk3, True)

            wfct = constp.tile([128, 64], f32)
            nc.sync.dma_start(out=wfct[:], in_=wfct_in.ap())
            bfc = constp.tile([1, 64], f32)
            nc.sync.dma_start(out=bfc[:], in_=bfc_in.ap())
            with tc.tile_pool(name="psF", bufs=1, space="PSUM") as psF:
                fc_ps = psF.tile([1, 64], f32, tag="fcps")
                nc.tensor.matmul(out=fc_ps[:], lhsT=xg[:], rhs=wfct[:],
                                 start=True, stop=True)
                ores = constp.tile([1, 64], f32)
                nc.vector.tensor_add(ores[:], fc_ps[:], bfc[:])
                nc.sync.dma_start(out=out_d.ap(), in_=ores[:])

    nc.compile()
    return nc


def _get_nc():
    if "nc" not in _cache:
        _cache["nc"] = _build()
    return _cache["nc"]


def _prep_inputs(x, W1, g1, b1, W2, g2, b2, W3, g3, b3, Wfc, bfc):
    def wuv(W, C):
        A, Bm = W[:, :C], W[:, C:]
        return np.concatenate([(A - Bm).T, Bm.T], axis=1).astype(np.float32)

    common = {
        "wuv1": wuv(np.asarray(W1), C0),
        "wuv2": wuv(np.asarray(W2), O1),
        "wuv3": wuv(np.asarray(W3), O2),
        "gb1": np.stack([g1, b1], 1).astype(np.float32),
        "gb2": np.stack([g2, b2], 1).astype(np.float32),
        "gb3": np.stack([g3, b3], 1).astype(np.float32),
        "wfct": np.asarray(Wfc).T.copy().astype(np.float32),
        "bfc": np.asarray(bfc)[None, :].astype(np.float32),
    }
    x = np.asarray(x, dtype=np.float32)
    return [{**common, "x": np.ascontiguousarray(x[c])} for c in range(NCORES)]


def _enable_jax_cache():
    try:
        import jax
        jax.config.update("jax_compilation_cache_dir", "/tmp/jaxcache")
        jax.config.update("jax_persistent_cache_min_entry_size_bytes", -1)
        jax.config.update("jax_persistent_cache_min_compile_time_secs", 0.5)
    except Exception:
        pass


def kernel(x, W1, g1, b1, W2, g2, b2, W3, g3, b3, Wfc, bfc):
    from concourse.bass_utils import run_bass_kernel_spmd
    _enable_jax_cache()
    nc = _get_nc()
    in_maps = _prep_inputs(x, W1, g1, b1, W2, g2, b2, W3, g3, b3, Wfc, bfc)
    res = run_bass_kernel_spmd(nc, in_maps, list(range(NCORES)))
    return np.stack([res.results[c]["out"][0] for c in range(NCORES)]).astype(np.float32)


# revision 4
# speedup vs baseline: 1.1426x; 1.1426x over previous
"""DGCNN feature extractor on 8 Trainium2 NeuronCores (Bass/Tile) — v3.

Data-parallel over batch B=8 (one sample per core). Per edge-conv layer:
  - scores s[n,m] = <x_n,x_m> - |x_m|^2/2 via PE matmul with appended aug
    rows (ones row on the query side, bias row on the key side)
  - scalar engine copies score chunks PSUM->SBUF; vector topk (chunked
    max8 -> top-16 -> 2x max_index) reads SBUF
  - neighbor fetch BATCHED: one SWDGE dma_gather per 4 point-tiles
    (8192 rows; 2 tiles for layer 3) from a DRAM [v | v^2] table
    (f32 for layers 1/2, bf16 for layer 3), wrapped-index tile built
    on-device with 8 PE selection matmuls + scalar shuffles + one PE
    broadcast matmul; fold trees lag one batch so vector never waits
  - BN batch stats via AllReduce; x_next = relu(a*(u + max_k v) + b)
Final: channel max pool, concat, FC on device; host stacks per-core rows.
"""
import numpy as np

B, C0, N, KNB = 8, 3, 4096, 16
O1, O2, O3 = 32, 32, 64
NCORES = 8
EPS = 1e-5
NTOT = float(B * N * KNB)
NT = N // 128          # 32 point-tiles per layer
NCH = 8                # score chunks per row (4096/512)
CHK = N // NCH         # 512
NEG = -3.0e38

_cache: dict = {}


def _build(sim_single=False):
    import concourse.bacc as bacc
    import concourse.bass as bass
    import concourse.mybir as mybir
    import concourse.tile as tile
    from concourse.masks import make_identity

    f32 = mybir.dt.float32
    bf16 = mybir.dt.bfloat16
    u32 = mybir.dt.uint32
    i16 = mybir.dt.int16
    AO = mybir.AluOpType
    AF = mybir.ActivationFunctionType

    nc = bacc.Bacc("TRN2", target_bir_lowering=False, debug=False,
                   num_devices=1 if sim_single else NCORES)

    # ---- I/O ----
    x_in = nc.dram_tensor("x", [C0, N], f32, kind="ExternalInput")
    wuv_in = [None,
              nc.dram_tensor("wuv1", [C0, 2 * O1], f32, kind="ExternalInput"),
              nc.dram_tensor("wuv2", [O1, 2 * O2], f32, kind="ExternalInput"),
              nc.dram_tensor("wuv3", [O2, 2 * O3], f32, kind="ExternalInput")]
    gb_in = [None,
             nc.dram_tensor("gb1", [O1, 2], f32, kind="ExternalInput"),
             nc.dram_tensor("gb2", [O2, 2], f32, kind="ExternalInput"),
             nc.dram_tensor("gb3", [O3, 2], f32, kind="ExternalInput")]
    wfct_in = nc.dram_tensor("wfct", [128, 64], f32, kind="ExternalInput")
    bfc_in = nc.dram_tensor("bfc", [1, 64], f32, kind="ExternalInput")
    out_d = nc.dram_tensor("out", [1, 64], f32, kind="ExternalOutput")

    # ---- internal DRAM ----
    vtab = [None,
            nc.dram_tensor("vtab1", [N, 2 * O1], f32),
            nc.dram_tensor("vtab2", [N, 2 * O2], f32),
            nc.dram_tensor("vtab3", [N, 2 * O3], bf16)]
    cc_in = [None] + [nc.dram_tensor(f"ccin{l}", [o, 2], f32)
                      for l, o in ((1, O1), (2, O2), (3, O3))]
    cc_out = [None] + [nc.dram_tensor(f"ccout{l}", [o, 2], f32,
                                      addr_space="Shared")
                       for l, o in ((1, O1), (2, O2), (3, O3))]

    with tile.TileContext(nc) as tc:
        with (
            tc.tile_pool(name="big", bufs=1) as bigp,
            tc.tile_pool(name="lay", bufs=1) as layp,
            tc.tile_pool(name="work", bufs=3) as workp,
            tc.tile_pool(name="gpool", bufs=2) as gp,
            tc.tile_pool(name="wip", bufs=2) as wip,
            tc.tile_pool(name="scb", bufs=2) as scbp,
            tc.tile_pool(name="const", bufs=1) as constp,
        ):
            ident = constp.tile([128, 128], f32)
            make_identity(nc, ident[:])
            ones128 = constp.tile([128, 1], f32)
            nc.vector.memset(ones128[:], 1.0)
            zero128 = constp.tile([128, 1], f32)
            nc.vector.memset(zero128[:], 0.0)
            eps128 = constp.tile([128, 1], f32)
            nc.vector.memset(eps128[:], EPS)
            xg = constp.tile([128, 1], f32)
            # BRD[q, 16b+qq] = (q == qq): replicates a [16,*] block 8x
            brd = constp.tile([16, 128], f32)
            for b in range(8):
                nc.vector.tensor_copy(out=brd[:, 16 * b:16 * b + 16],
                                      in_=ident[0:16, 0:16])

            def layer(l, C, O, xq, xk, is_last):
                """xq/xk: [C+1, N] SBUF tiles, rows 0..C-1 = x, row C = aug."""
                BT = 4          # tiles per gather batch
                vdt = bf16 if l == 3 else f32
                wuv = constp.tile([C, 2 * O], f32, tag=f"wuv{l}")
                nc.sync.dma_start(out=wuv[:], in_=wuv_in[l].ap())
                gb = constp.tile([O, 2], f32, tag=f"gb{l}")
                nc.sync.dma_start(out=gb[:], in_=gb_in[l].ap())

                # --- phase A: aug rows, u/v matmuls, vtab ---
                augst = layp.tile([1, N], f32, tag="augst")
                nc.vector.memset(augst[:], 1.0)
                nc.sync.dma_start(out=xq[C:C + 1, :], in_=augst[:])
                sqst = augst
                xsq = layp.tile([C, N], f32, tag="xsq")
                nc.scalar.activation(out=xsq[:], in_=xq[0:C, :],
                                     func=AF.Square, bias=zero128[0:C, :])
                onesC = constp.tile([C, 1], f32, tag=f"onesC{l}")
                nc.vector.memset(onesC[:], 1.0)
                ubuf = layp.tile([128, NT, O], f32, tag="ubuf")
                with tc.tile_pool(name=f"psA{l}", bufs=3, space="PSUM") as psA:
                    for ch in range(8):
                        sl = slice(512 * ch, 512 * (ch + 1))
                        sq_ps = psA.tile([1, 512], f32, tag="sqps")
                        nc.tensor.matmul(out=sq_ps[:], lhsT=onesC[:],
                                         rhs=xsq[:, sl], start=True, stop=True)
                        nc.scalar.activation(out=sqst[:, sl], in_=sq_ps[:],
                                             func=AF.Copy, scale=-0.5)
                    nc.sync.dma_start(out=xk[C:C + 1, :], in_=sqst[:])
                    for t in range(NT):
                        tl = slice(128 * t, 128 * (t + 1))
                        uv_ps = psA.tile([128, 2 * O], f32, tag="uvps")
                        nc.tensor.matmul(out=uv_ps[:], lhsT=xq[0:C, tl],
                                         rhs=wuv[:], start=True, stop=True)
                        nc.scalar.activation(out=ubuf[:, t, :], in_=uv_ps[:, 0:O],
                                             func=AF.Copy)
                        vstage = workp.tile([128, 2 * O], vdt, tag="vstage")
                        nc.vector.tensor_copy(out=vstage[:, 0:O], in_=uv_ps[:, O:2 * O])
                        nc.scalar.activation(out=vstage[:, O:2 * O],
                                             in_=uv_ps[:, O:2 * O],
                                             func=AF.Square, bias=zero128[:, :])
                        nc.sync.dma_start(out=vtab[l].ap()[tl, :], in_=vstage[:])

                # --- phase B ---
                Dbuf = layp.tile([128, NT, O], f32, tag="Dbuf")
                GG = layp.tile([128, NT, 2 * O], f32, tag="GG")
                with (
                    tc.tile_pool(name=f"psB{l}", bufs=1, space="PSUM") as psB,
                    tc.tile_pool(name=f"psW{l}", bufs=2, space="PSUM") as psW,
                ):
                    def folds_for(bt, BTn, g4):
                        """reduce over k for a whole gather batch via strided APs.
                        g4: [128, BTn*16, 2*O]; writes Dbuf/GG tile rows."""
                        base = g4[:]
                        kv = bass.AP(base.tensor, base.offset,
                                     [base.ap[0], [16 * 2 * O, BTn], [1, O],
                                      [2 * O, 16]])
                        nc.vector.tensor_reduce(
                            out=Dbuf[:, bt * BTn:(bt + 1) * BTn, :], in_=kv,
                            axis=mybir.AxisListType.X, op=AO.max)
                        ks = bass.AP(base.tensor, base.offset,
                                     [base.ap[0], [16 * 2 * O, BTn], [1, 2 * O],
                                      [2 * O, 16]])
                        nc.vector.tensor_reduce(
                            out=GG[:, bt * BTn:(bt + 1) * BTn, :], in_=ks,
                            axis=mybir.AxisListType.X, op=AO.add)

                    pend = None     # (batch_start_tile, g4)
                    nbt = NT // BT
                    for bt in range(nbt):
                        idxs4 = wip.tile([128, BT * 16], u32, tag="idxs4")
                        for u in range(BT):
                            t = bt * BT + u
                            tl = slice(128 * t, 128 * (t + 1))
                            ssb = scbp.tile([128, N], f32, tag="ssb")
                            for h in range(2):
                                stile = psB.tile([128, 2048], f32, tag="stile")
                                for c4 in range(4):
                                    sl = slice(2048 * h + 512 * c4,
                                               2048 * h + 512 * (c4 + 1))
                                    nc.tensor.matmul(
                                        out=stile[:, 512 * c4:512 * (c4 + 1)],
                                        lhsT=xq[:, tl], rhs=xk[:, sl],
                                        start=True, stop=True)
                                nc.scalar.activation(
                                    out=ssb[:, 2048 * h:2048 * (h + 1)],
                                    in_=stile[:], func=AF.Copy)
                            cand = workp.tile([128, 8 * NCH], f32, tag="cand")
                            for ch in range(NCH):
                                nc.vector.max(out=cand[:, 8 * ch:8 * ch + 8],
                                              in_=ssb[:, CHK * ch:CHK * (ch + 1)])
                            t16 = workp.tile([128, 16], f32, tag="t16")
                            cand2 = workp.tile([128, 8 * NCH], f32, tag="cand2")
                            nc.vector.max(out=t16[:, 0:8], in_=cand[:])
                            nc.vector.match_replace(out=cand2[:],
                                                    in_to_replace=t16[:, 0:8],
                                                    in_values=cand[:], imm_value=NEG)
                            nc.vector.max(out=t16[:, 8:16], in_=cand2[:])
                            nc.vector.max_index(out=idxs4[:, 16 * u:16 * u + 8],
                                                in_max=t16[:, 0:8], in_values=ssb[:])
                            nc.vector.max_index(out=idxs4[:, 16 * u + 8:16 * u + 16],
                                                in_max=t16[:, 8:16], in_values=ssb[:])
                        # batched wrapped-index:
                        # wi[16b+q, 128u+8j+r] = idxs4[16r+q, 16u+j]
                        idxf4 = wip.tile([128, BT * 16], f32, tag="idxf4")
                        nc.vector.tensor_copy(out=idxf4[:], in_=idxs4[:])
                        mps4 = psW.tile([16, 8, BT * 16], f32, tag="mps4")
                        for r in range(8):
                            nc.tensor.matmul(out=mps4[:, r, :],
                                             lhsT=ident[:, 16 * r:16 * r + 16],
                                             rhs=idxf4[:], start=True, stop=True)
                        w16 = wip.tile([16, BT, 16, 8], f32, tag="w16")
                        for r in range(8):
                            nc.scalar.activation(out=w16[:, :, :, r],
                                                 in_=mps4[:, r, :], func=AF.Copy)
                        rep4 = psW.tile([128, BT * 128], f32, tag="rep4")
                        nc.tensor.matmul(out=rep4[:], lhsT=brd[:],
                                         rhs=w16[:, :, :, :], start=True, stop=True)
                        wi4 = wip.tile([128, BT * 128], i16, tag="wi4")
                        nc.vector.tensor_copy(out=wi4[:], in_=rep4[:])
                        g4 = gp.tile([128, BT * 16, 2 * O], vdt, tag="g4")
                        nc.gpsimd.dma_gather(g4[:], vtab[l].ap(), wi4[:],
                                             BT * 2048, BT * 2048, 2 * O,
                                             single_packet=False)
                        if pend is not None:
                            folds_for(pend[0], BT, pend[1])
                        pend = (bt, g4)
                    folds_for(pend[0], BT, pend[1])

                # --- phase C: stats, allreduce, x_next ---
                tmp2 = layp.tile([128, NT, O], f32, tag="tmp2")
                nc.vector.tensor_tensor(out=tmp2[:], in0=ubuf[:], in1=GG[:, :, 0:O],
                                        op=AO.mult)
                nc.vector.tensor_scalar(tmp2[:], tmp2[:], 2.0, None, op0=AO.mult)
                nc.vector.tensor_tensor(out=GG[:, :, O:2 * O], in0=GG[:, :, O:2 * O],
                                        in1=tmp2[:], op=AO.add)
                nc.vector.tensor_tensor(out=tmp2[:], in0=ubuf[:], in1=ubuf[:],
                                        op=AO.mult)
                nc.vector.tensor_scalar(tmp2[:], tmp2[:], 16.0, None, op0=AO.mult)
                nc.vector.tensor_tensor(out=GG[:, :, O:2 * O], in0=GG[:, :, O:2 * O],
                                        in1=tmp2[:], op=AO.add)
                nc.vector.tensor_scalar(tmp2[:], ubuf[:], 16.0, None, op0=AO.mult)
                nc.vector.tensor_tensor(out=GG[:, :, 0:O], in0=GG[:, :, 0:O],
                                        in1=tmp2[:], op=AO.add)
                for h in (16, 8, 4, 2, 1):
                    nc.vector.tensor_tensor(out=GG[:, 0:h, :], in0=GG[:, 0:h, :],
                                            in1=GG[:, h:2 * h, :], op=AO.add)
                with tc.tile_pool(name=f"psR{l}", bufs=1, space="PSUM") as psR:
                    s1_ps = psR.tile([O, 1], f32, tag="s1ps")
                    s2_ps = psR.tile([O, 1], f32, tag="s2ps")
                    nc.tensor.matmul(out=s1_ps[:], lhsT=GG[:, 0, 0:O], rhs=ones128[:],
                                     start=True, stop=True)
                    nc.tensor.matmul(out=s2_ps[:], lhsT=GG[:, 0, O:2 * O],
                                     rhs=ones128[:], start=True, stop=True)
                    stg = workp.tile([O, 2], f32, tag="stg")
                    nc.vector.tensor_copy(out=stg[:, 0:1], in_=s1_ps[:])
                    nc.vector.tensor_copy(out=stg[:, 1:2], in_=s2_ps[:])
                    nc.sync.dma_start(out=cc_in[l].ap(), in_=stg[:])
                if sim_single:
                    nc.sync.dma_start(out=cc_out[l].ap(), in_=cc_in[l].ap())
                else:
                    nc.gpsimd.collective_compute(
                        "AllReduce", AO.add, replica_groups=[list(range(NCORES))],
                        ins=[cc_in[l].ap()], outs=[cc_out[l].ap()])
                stats = workp.tile([O, 2], f32, tag="stats")
                nc.sync.dma_start(out=stats[:], in_=cc_out[l].ap())
                mean = workp.tile([O, 4], f32, tag="mean")
                nc.vector.tensor_scalar(mean[:, 0:1], stats[:, 0:1], 1.0 / NTOT,
                                        None, op0=AO.mult)
                nc.vector.tensor_scalar(mean[:, 1:2], stats[:, 1:2], 1.0 / NTOT,
                                        None, op0=AO.mult)
                nc.vector.tensor_tensor(out=mean[:, 2:3], in0=mean[:, 0:1],
                                        in1=mean[:, 0:1], op=AO.mult)
                nc.vector.tensor_sub(mean[:, 1:2], mean[:, 1:2], mean[:, 2:3])
                nc.scalar.activation(out=mean[:, 1:2], in_=mean[:, 1:2],
                                     func=AF.Sqrt, bias=eps128[0:O, :])
                ab = workp.tile([O, 2], f32, tag="ab")
                nc.vector.reciprocal(out=ab[:, 0:1], in_=mean[:, 1:2])
                nc.vector.tensor_tensor(out=ab[:, 0:1], in0=ab[:, 0:1],
                                        in1=gb[:, 0:1], op=AO.mult)
                nc.vector.tensor_tensor(out=mean[:, 3:4], in0=mean[:, 0:1],
                                        in1=ab[:, 0:1], op=AO.mult)
                nc.vector.tensor_sub(ab[:, 1:2], gb[:, 1:2], mean[:, 3:4])

                nc.vector.tensor_tensor(out=Dbuf[:], in0=Dbuf[:], in1=ubuf[:],
                                        op=AO.add)
                if is_last:
                    xnq = bigp.tile([O3 + 1, N], f32, tag="xq")
                    xnk = None
                else:
                    xnq = bigp.tile([O + 1, N], f32, tag="xq")
                    xnk = bigp.tile([O + 1, N], f32, tag="xk")
                with tc.tile_pool(name=f"psT{l}", bufs=4, space="PSUM") as psT:
                    for t in range(NT):
                        tl = slice(128 * t, 128 * (t + 1))
                        tp = psT.tile([O, 128], f32, tag="tpps")
                        nc.tensor.transpose(out=tp[:], in_=Dbuf[:, t, :],
                                            identity=ident[:])
                        nc.scalar.activation(out=xnq[0:O, tl], in_=tp[:],
                                             func=AF.Relu,
                                             bias=ab[:, 1:2], scale=ab[:, 0:1])
                if xnk is not None:
                    nc.scalar.activation(out=xnk[0:O, :], in_=xnq[0:O, :],
                                         func=AF.Copy)
                cm = workp.tile([O, 1], f32, tag="cm")
                nc.vector.tensor_reduce(out=cm[:], in_=xnq[0:O, :],
                                        axis=mybir.AxisListType.X, op=AO.max)
                off = {1: 0, 2: O1, 3: O1 + O2}[l]
                nc.sync.dma_start(out=xg[off:off + O, :], in_=cm[:])
                return xnq, xnk

            xq1 = bigp.tile([C0 + 1, N], f32, tag="xq")
            xk1 = bigp.tile([C0 + 1, N], f32, tag="xk")
            nc.sync.dma_start(out=xq1[0:C0, :], in_=x_in.ap())
            nc.vector.tensor_copy(out=xk1[0:C0, :], in_=xq1[0:C0, :])

            xq2, xk2 = layer(1, C0, O1, xq1, xk1, False)
            xq3, xk3 = layer(2, O1, O2, xq2, xk2, False)
            layer(3, O2, O3, xq3, xk3, True)

            wfct = constp.tile([128, 64], f32)
            nc.sync.dma_start(out=wfct[:], in_=wfct_in.ap())
            bfc = constp.tile([1, 64], f32)
            nc.sync.dma_start(out=bfc[:], in_=bfc_in.ap())
            with tc.tile_pool(name="psF", bufs=1, space="PSUM") as psF:
                fc_ps = psF.tile([1, 64], f32, tag="fcps")
                nc.tensor.matmul(out=fc_ps[:], lhsT=xg[:], rhs=wfct[:],
                                 start=True, stop=True)
                ores = constp.tile([1, 64], f32)
                nc.vector.tensor_add(ores[:], fc_ps[:], bfc[:])
                nc.sync.dma_start(out=out_d.ap(), in_=ores[:])

    nc.compile()
    return nc


def _get_nc():
    if "nc" not in _cache:
        _cache["nc"] = _build()
    return _cache["nc"]


def _prep_inputs(x, W1, g1, b1, W2, g2, b2, W3, g3, b3, Wfc, bfc):
    def wuv(W, C):
        A, Bm = W[:, :C], W[:, C:]
        return np.concatenate([(A - Bm).T, Bm.T], axis=1).astype(np.float32)

    common = {
        "wuv1": wuv(np.asarray(W1), C0),
        "wuv2": wuv(np.asarray(W2), O1),
        "wuv3": wuv(np.asarray(W3), O2),
        "gb1": np.stack([g1, b1], 1).astype(np.float32),
        "gb2": np.stack([g2, b2], 1).astype(np.float32),
        "gb3": np.stack([g3, b3], 1).astype(np.float32),
        "wfct": np.asarray(Wfc).T.copy().astype(np.float32),
        "bfc": np.asarray(bfc)[None, :].astype(np.float32),
    }
    x = np.asarray(x, dtype=np.float32)
    return [{**common, "x": np.ascontiguousarray(x[c])} for c in range(NCORES)]


def _enable_jax_cache():
    try:
        import jax
        jax.config.update("jax_compilation_cache_dir", "/tmp/jaxcache")
        jax.config.update("jax_persistent_cache_min_entry_size_bytes", -1)
        jax.config.update("jax_persistent_cache_min_compile_time_secs", 0.5)
    except Exception:
        pass


def kernel(x, W1, g1, b1, W2, g2, b2, W3, g3, b3, Wfc, bfc):
    from concourse.bass_utils import run_bass_kernel_spmd
    _enable_jax_cache()
    nc = _get_nc()
    in_maps = _prep_inputs(x, W1, g1, b1, W2, g2, b2, W3, g3, b3, Wfc, bfc)
    res = run_bass_kernel_spmd(nc, in_maps, list(range(NCORES)))
    return np.stack([res.results[c]["out"][0] for c in range(NCORES)]).astype(np.float32)
